# revision 11
# baseline (speedup 1.0000x reference)
"""Trainium2 Bass kernel for a dense transformer block (LN1 -> 16-head causal
attention -> residual -> LN2 -> SwiGLU FFN -> residual).

Full shapes: x [4, 2048, 1024], H=16 heads of DH=64, FFN hidden 4096, fp32 IO.

Sharding (8 NeuronCores): core c handles batch b = c//2 and head-group
hg = c%2 (8 of the 16 heads). Each core computes LN1 + Q/K/V + causal
attention + its partial output projection for ALL 2048 tokens of its batch,
then a 2-core ReduceScatter over the pair {2b, 2b+1} sums the partial
attention outputs and hands each core a 1024-token half. Each core finishes
residual + LN2 + SwiGLU FFN + residual for its token half. No other
communication. Compute is done in bf16 on the TensorEngine with fp32 PSUM
accumulation; residuals stay fp32. LN gammas are folded into the following
matmul weights on the host (betas are zero for this problem).
"""
import sys, math

sys.path.insert(0, "/opt/trn_rl_repo")

import numpy as np
import ml_dtypes

import concourse.bacc as bacc
import concourse.bass as bass
import concourse.tile as tile
from concourse import mybir
from concourse.bass_utils import run_bass_kernel_spmd

BF16 = ml_dtypes.bfloat16
F32 = mybir.dt.float32
BF = mybir.dt.bfloat16
EPS = 1e-5


class Geom:
    def __init__(self, S=2048, E=1024, H=16, DH=64, DF=4096, B=4, n_cores=8):
        self.S, self.E, self.H, self.DH, self.DF, self.B = S, E, H, DH, DF, B
        self.n_cores = n_cores
        self.SH = S // 2              # local token half
        self.EK = E // 128            # e contraction tiles
        self.HL = H // 2              # local heads per core
        self.HP = self.HL // 2        # local head pairs
        self.HD = self.HL * DH        # local concat head dim (512)
        self.NDF = DF // 128          # FFN hidden tiles
        self.NQT = S // 128           # token tiles (full seq)
        self.NQG = max(1, S // 512)   # attention query groups (512 wide)
        self.QG = S // self.NQG      # query group width (<=512)
        self.NLT = self.SH // 128     # local token tiles
        self.NE = max(1, E // 512)    # 512-wide chunks of E
        self.EC = E // self.NE       # E chunk width
        assert self.QG % 128 == 0 and self.HD == 512
        self.KT_PER_QG = self.QG // 128
        self.pairs = [[2 * i, 2 * i + 1] for i in range(n_cores // 2)]


FULL = Geom()


def build_program(g: Geom):
    nc = bacc.Bacc("TRN2", target_bir_lowering=False, debug=False,
                   enable_asserts=True, num_devices=g.n_cores)

    x_full = nc.dram_tensor("x_full", [g.S, g.E], F32, kind="ExternalInput")
    x_res = nc.dram_tensor("x_res", [g.SH, g.E], F32, kind="ExternalInput")
    wq_d = nc.dram_tensor("wq", [128, g.EK, g.HD], BF, kind="ExternalInput")
    wk_d = nc.dram_tensor("wk", [128, g.EK, g.HD], BF, kind="ExternalInput")
    wv_d = nc.dram_tensor("wv", [128, g.EK, g.HD], BF, kind="ExternalInput")
    wo_d = nc.dram_tensor("wo", [128, g.HD // 128, g.E], BF, kind="ExternalInput")
    w1_d = nc.dram_tensor("w1", [g.NDF, 128, g.EK, 128], BF, kind="ExternalInput")
    w2_d = nc.dram_tensor("w2", [g.NDF, 128, g.EK, 128], BF, kind="ExternalInput")
    w3_d = nc.dram_tensor("w3", [g.NDF, 128, g.E], BF, kind="ExternalInput")
    b1_d = nc.dram_tensor("b1", [128, g.NDF], F32, kind="ExternalInput")
    b2_d = nc.dram_tensor("b2", [128, g.NDF], F32, kind="ExternalInput")
    boh_d = nc.dram_tensor("bo_half", [1, g.E], BF, kind="ExternalInput")
    b3_d = nc.dram_tensor("b3", [1, g.E], BF, kind="ExternalInput")
    id_d = nc.dram_tensor("ident", [128, 128], BF, kind="ExternalInput")
    out_d = nc.dram_tensor("out", [g.SH, g.E], F32, kind="ExternalOutput")

    scale = 1.0 / math.sqrt(g.DH)
    is_ge = mybir.AluOpType.is_ge

    with tile.TileContext(nc) as tc:
        with (
            tc.tile_pool(name="persist", bufs=1) as P0,
            tc.tile_pool(name="dram", bufs=1, space="DRAM") as dram,
        ):
            # ---- whole-kernel persistents -----------------------------------
            r1 = P0.tile([128, g.NLT, g.E], F32, name="r1")
            h2T = P0.tile([128, g.EK, g.SH], BF, name="h2T")
            b1_t = P0.tile([128, g.NDF], F32, name="b1_t")
            b2_t = P0.tile([128, g.NDF], F32, name="b2_t")
            boh_t = P0.tile([1, g.E], BF, name="boh_t")
            b3_t = P0.tile([1, g.E], BF, name="b3_t")
            id_t = P0.tile([128, 128], BF, name="id_t")
            ones_t = P0.tile([1, 128], BF, name="ones_t")
            eps_t = P0.tile([128, 1], F32, name="eps_t")
            nc.vector.memset(eps_t[:], EPS)
            nc.sync.dma_start(b1_t[:], b1_d[:])
            nc.sync.dma_start(b2_t[:], b2_d[:])
            nc.sync.dma_start(boh_t[:], boh_d[:])
            nc.sync.dma_start(b3_t[:], b3_d[:])
            nc.sync.dma_start(id_t[:], id_d[:])
            nc.vector.memset(ones_t[:], 1.0)

            ao_bounce = dram.tile([g.S, g.E], F32)
            ao_red = dram.tile([g.SH, g.E], F32)

            # ---- attention era ----------------------------------------------
            with tc.tile_pool(name="attn_era", bufs=1) as P1:
                QT = P1.tile([128, g.HP, g.S], BF, name="QT")
                KT = P1.tile([128, g.HP, g.S], BF, name="KT")
                V_aug = P1.tile([128, g.NQT, g.HL, 65], BF, name="V_aug")
                ctx_sb = P1.tile([128, g.NQT, g.HL, g.DH], BF, name="ctx_sb")
                ctxT = P1.tile([128, g.HD // 128, g.S], BF, name="ctxT")
                wo_t = P1.tile([128, g.HD // 128, g.E], BF, name="wo_t")
                nc.sync.dma_start(wo_t[:], wo_d[:])

                # Phase A+B: LN1, transpose, Q/K/V per 128-token tile
                with (
                    tc.tile_pool(name="ab", bufs=3) as AB,
                    tc.tile_pool(name="abw", bufs=1) as ABW,
                    tc.tile_pool(name="ab_ps_tr", bufs=2, space="PSUM") as PStr,
                    tc.tile_pool(name="ab_ps_qk", bufs=2, space="PSUM") as PSqk,
                    tc.tile_pool(name="ab_ps_v", bufs=2, space="PSUM") as PSv,
                ):
                    wq_t = ABW.tile([128, g.EK, g.HD], BF, name="wq_t")
                    wk_t = ABW.tile([128, g.EK, g.HD], BF, name="wk_t")
                    wv_t = ABW.tile([128, g.EK, g.HD], BF, name="wv_t")
                    nc.sync.dma_start(wq_t[:], wq_d[:])
                    nc.sync.dma_start(wk_t[:], wk_d[:])
                    nc.sync.dma_start(wv_t[:], wv_d[:])

                    for qt in range(g.NQT):
                        x_t = AB.tile([128, g.E], F32, name="x_t")
                        nc.sync.dma_start(x_t[:], x_full[qt * 128:(qt + 1) * 128, :])
                        bn6 = AB.tile([128, g.NE, 6], F32, name="bn6")
                        for ch in range(g.NE):
                            nc.vector.bn_stats(bn6[:, ch, :], x_t[:, ch * g.EC:(ch + 1) * g.EC])
                        mv = AB.tile([128, 2], F32, name="mv")
                        nc.vector.bn_aggr(mv[:], bn6[:])
                        std = AB.tile([128, 1], F32, name="std")
                        nc.scalar.activation(std[:], mv[:, 1:2],
                                             mybir.ActivationFunctionType.Sqrt, bias=eps_t[:])
                        rstd = AB.tile([128, 1], F32, name="rstd")
                        nc.vector.reciprocal(rstd[:], std[:])
                        h_t = AB.tile([128, g.E], BF, name="h_t")
                        nc.vector.tensor_scalar(h_t[:], x_t[:], mv[:, 0:1], rstd[:],
                                                mybir.AluOpType.subtract,
                                                mybir.AluOpType.mult)
                        hT_t = AB.tile([128, g.EK, 128], BF, name="hT_t")
                        for ek in range(g.EK):
                            tp = PStr.tile([128, 128], BF, name="tp")
                            nc.tensor.transpose(tp[:], h_t[:, ek * 128:(ek + 1) * 128], id_t[:])
                            nc.vector.tensor_copy(hT_t[:, ek, :], tp[:])
                        # V token-major (+ ones column)
                        pv = PSv.tile([128, g.HD], F32, name="pv")
                        for ek in range(g.EK):
                            nc.tensor.matmul(pv[:], hT_t[:, ek, :], wv_t[:, ek, :],
                                             start=(ek == 0), stop=(ek == g.EK - 1))
                        nc.scalar.copy(V_aug[:, qt, :, 0:g.DH],
                                       pv[:].rearrange("p (h d) -> p h d", h=g.HL))
                        nc.vector.memset(V_aug[:, qt, :, g.DH:g.DH + 1], 1.0)
                        # Q/K feature-major per head pair
                        for hp in range(g.HP):
                            pq = PSqk.tile([128, 128], F32, name="pq")
                            for ek in range(g.EK):
                                nc.tensor.matmul(pq[:], wq_t[:, ek, hp * 128:(hp + 1) * 128],
                                                 hT_t[:, ek, :],
                                                 start=(ek == 0), stop=(ek == g.EK - 1))
                            nc.scalar.copy(QT[:, hp, qt * 128:(qt + 1) * 128], pq[:])
                            pk = PSqk.tile([128, 128], F32, name="pk")
                            for ek in range(g.EK):
                                nc.tensor.matmul(pk[:], wk_t[:, ek, hp * 128:(hp + 1) * 128],
                                                 hT_t[:, ek, :],
                                                 start=(ek == 0), stop=(ek == g.EK - 1))
                            nc.scalar.copy(KT[:, hp, qt * 128:(qt + 1) * 128], pk[:])

                # Phase C: causal attention per (head, query group)
                with (
                    tc.tile_pool(name="cpb", bufs=6) as CP,
                    tc.tile_pool(name="c_ps_s", bufs=3, space="PSUM") as PSs,
                    tc.tile_pool(name="c_ps_c", bufs=1, space="PSUM") as PSc,
                ):
                    for h in range(g.HL):
                        hp, dp = h // 2, (h % 2) * g.DH
                        for qg in range(g.NQG):
                            nkt = g.KT_PER_QG * (qg + 1)
                            pcs = [PSc.tile([128, 65], F32, name=f"pc{qs}")
                                   for qs in range(g.QG // 128)]
                            for kt in range(nkt):
                                ps = PSs.tile([128, g.QG], F32, name="ps")
                                nc.tensor.matmul(ps[:],
                                                 KT[dp:dp + g.DH, hp, kt * 128:(kt + 1) * 128],
                                                 QT[dp:dp + g.DH, hp, qg * g.QG:(qg + 1) * g.QG],
                                                 start=True, stop=True)
                                pb = CP.tile([128, g.QG], BF, name="pb")
                                nc.scalar.activation(pb[:], ps[:],
                                                     mybir.ActivationFunctionType.Exp,
                                                     scale=scale)
                                if kt >= nkt - g.KT_PER_QG:
                                    # zero strictly-upper (k > q) entries:
                                    # keep where (q - k) >= 0
                                    nc.gpsimd.affine_select(
                                        pb[:], pb[:], [[1, g.QG]], is_ge, 0.0,
                                        base=qg * g.QG - kt * 128,
                                        channel_multiplier=-1)
                                for qs in range(g.QG // 128):
                                    nc.tensor.matmul(pcs[qs][:],
                                                     pb[:, qs * 128:(qs + 1) * 128],
                                                     V_aug[:, kt, h, :],
                                                     start=(kt == 0), stop=(kt == nkt - 1))
                            for qs in range(g.QG // 128):
                                qt = qg * (g.QG // 128) + qs
                                rec = CP.tile([128, 1], F32, name="rec")
                                nc.vector.reciprocal(rec[:], pcs[qs][:, g.DH:g.DH + 1])
                                nc.vector.tensor_scalar(ctx_sb[:, qt, h, :],
                                                        pcs[qs][:, 0:g.DH], rec[:], None,
                                                        mybir.AluOpType.mult)

                # Phase D: transpose ctx to feature-major
                with tc.tile_pool(name="d_ps", bufs=3, space="PSUM") as PSd:
                    for c in range(g.HD // 128):
                        for qt in range(g.NQT):
                            tp2 = PSd.tile([128, 128], BF, name="tp2")
                            nc.tensor.transpose(tp2[:], ctx_sb[:, qt, 2 * c:2 * c + 2, :], id_t[:])
                            nc.vector.tensor_copy(ctxT[:, c, qt * 128:(qt + 1) * 128], tp2[:])

                # Phase E: partial attention output projection (+ bo/2), to DRAM
                with (
                    tc.tile_pool(name="e_sb", bufs=3) as EB,
                    tc.tile_pool(name="e_ps", bufs=3, space="PSUM") as PSe,
                ):
                    for qt in range(g.NQT):
                        for ne in range(g.NE):
                            pa = PSe.tile([128, g.EC], F32, name="pa")
                            for c in range(g.HD // 128):
                                nc.tensor.matmul(pa[:], ctxT[:, c, qt * 128:(qt + 1) * 128],
                                                 wo_t[:, c, ne * g.EC:(ne + 1) * g.EC],
                                                 start=(c == 0), stop=False)
                            nc.tensor.matmul(pa[:], ones_t[:, 0:128],
                                             boh_t[:, ne * g.EC:(ne + 1) * g.EC],
                                             start=False, stop=True)
                            ao_t = EB.tile([128, g.EC], F32, name="ao_t")
                            nc.scalar.copy(ao_t[:], pa[:])
                            nc.sync.dma_start(
                                ao_bounce[qt * 128:(qt + 1) * 128, ne * g.EC:(ne + 1) * g.EC],
                                ao_t[:])

            # Phase F: pairwise ReduceScatter of partial attention outputs
            nc.gpsimd.collective_compute(
                "ReduceScatter", mybir.AluOpType.add,
                replica_groups=g.pairs,
                ins=[ao_bounce.opt()], outs=[ao_red.opt()])

            # Phase G: residual + LN2 + transpose (local token half)
            with (
                tc.tile_pool(name="g_sb", bufs=3) as GB,
                tc.tile_pool(name="g_ps", bufs=3, space="PSUM") as PSg,
            ):
                for lt in range(g.NLT):
                    xr = GB.tile([128, g.E], F32, name="xr")
                    nc.sync.dma_start(xr[:], x_res[lt * 128:(lt + 1) * 128, :])
                    ar = GB.tile([128, g.E], F32, name="ar")
                    nc.sync.dma_start(ar[:], ao_red[lt * 128:(lt + 1) * 128, :])
                    nc.vector.tensor_add(r1[:, lt, :], xr[:], ar[:])
                    bn6b = GB.tile([128, g.NE, 6], F32, name="bn6b")
                    for ch in range(g.NE):
                        nc.vector.bn_stats(bn6b[:, ch, :], r1[:, lt, ch * g.EC:(ch + 1) * g.EC])
                    mvb = GB.tile([128, 2], F32, name="mvb")
                    nc.vector.bn_aggr(mvb[:], bn6b[:])
                    stdb = GB.tile([128, 1], F32, name="stdb")
                    nc.scalar.activation(stdb[:], mvb[:, 1:2],
                                         mybir.ActivationFunctionType.Sqrt, bias=eps_t[:])
                    rstdb = GB.tile([128, 1], F32, name="rstdb")
                    nc.vector.reciprocal(rstdb[:], stdb[:])
                    h2_t = GB.tile([128, g.E], BF, name="h2_t")
                    nc.vector.tensor_scalar(h2_t[:], r1[:, lt, :], mvb[:, 0:1], rstdb[:],
                                            mybir.AluOpType.subtract, mybir.AluOpType.mult)
                    for ek in range(g.EK):
                        tp3 = PSg.tile([128, 128], BF, name="tp3")
                        nc.tensor.transpose(tp3[:], h2_t[:, ek * 128:(ek + 1) * 128], id_t[:])
                        nc.vector.tensor_copy(h2T[:, ek, lt * 128:(lt + 1) * 128], tp3[:])

            # Phase H: SwiGLU FFN
            n_hc = max(1, g.SH // 512)   # 512-wide chunks of the local tokens
            HC = g.SH // n_hc
            with tc.tile_pool(name="h_era", bufs=1) as P3:
                gsT = P3.tile([128, g.NDF, g.SH], BF, name="gsT")
                with (
                    tc.tile_pool(name="h1_sb", bufs=3) as H1B,
                    tc.tile_pool(name="h1w", bufs=3) as H1W,
                    tc.tile_pool(name="h1_ps_a", bufs=2, space="PSUM") as PSa,
                    tc.tile_pool(name="h1_ps_g", bufs=2, space="PSUM") as PSgg,
                ):
                    for df in range(g.NDF):
                        w1_t = H1W.tile([128, g.EK, 128], BF, name="w1_t")
                        nc.sync.dma_start(w1_t[:], w1_d[df])
                        w2_t = H1W.tile([128, g.EK, 128], BF, name="w2_t")
                        nc.sync.dma_start(w2_t[:], w2_d[df])
                        for hc in range(n_hc):
                            sl = slice(hc * HC, (hc + 1) * HC)
                            pa1 = PSa.tile([128, HC], F32, name="pa1")
                            for ek in range(g.EK):
                                nc.tensor.matmul(pa1[:], w1_t[:, ek, :], h2T[:, ek, sl],
                                                 start=(ek == 0), stop=(ek == g.EK - 1))
                            pg1 = PSgg.tile([128, HC], F32, name="pg1")
                            for ek in range(g.EK):
                                nc.tensor.matmul(pg1[:], w2_t[:, ek, :], h2T[:, ek, sl],
                                                 start=(ek == 0), stop=(ek == g.EK - 1))
                            sg = H1B.tile([128, HC], BF, name="sg")
                            nc.scalar.activation(sg[:], pa1[:],
                                                 mybir.ActivationFunctionType.Sigmoid,
                                                 bias=b1_t[:, df:df + 1])
                            sa = H1B.tile([128, HC], BF, name="sa")
                            nc.vector.scalar_tensor_tensor(sa[:], pa1[:],
                                                           b1_t[:, df:df + 1], sg[:],
                                                           mybir.AluOpType.add,
                                                           mybir.AluOpType.mult)
                            nc.vector.scalar_tensor_tensor(gsT[:, df, sl], pg1[:],
                                                           b2_t[:, df:df + 1], sa[:],
                                                           mybir.AluOpType.add,
                                                           mybir.AluOpType.mult)
                # H2: down projection + final residual
                with (
                    tc.tile_pool(name="h2_sb", bufs=3) as H2B,
                    tc.tile_pool(name="h2_w3", bufs=1) as H2W,
                    tc.tile_pool(name="h2_ps", bufs=3, space="PSUM") as PSf,
                ):
                    for ne in range(g.NE):
                        w3_t = H2W.tile([128, g.NDF, g.EC], BF, name="w3_t")
                        nc.sync.dma_start(w3_t[:], w3_d[:, :, ne * g.EC:(ne + 1) * g.EC]
                                          .rearrange("a b c -> b a c"))
                        for lt in range(g.NLT):
                            pf = PSf.tile([128, g.EC], F32, name="pf")
                            for df in range(g.NDF):
                                nc.tensor.matmul(pf[:], gsT[:, df, lt * 128:(lt + 1) * 128],
                                                 w3_t[:, df, :],
                                                 start=(df == 0), stop=False)
                            nc.tensor.matmul(pf[:], ones_t[:, 0:128],
                                             b3_t[:, ne * g.EC:(ne + 1) * g.EC],
                                             start=False, stop=True)
                            o_t = H2B.tile([128, g.EC], F32, name="o_t")
                            nc.vector.tensor_add(o_t[:], pf[:],
                                                 r1[:, lt, ne * g.EC:(ne + 1) * g.EC])
                            nc.sync.dma_start(
                                out_d[lt * 128:(lt + 1) * 128, ne * g.EC:(ne + 1) * g.EC],
                                o_t[:])

    nc.compile()
    return nc


def make_in_maps(g: Geom, x, wq, wk, wv, wo, bo, w1, b1, w2, b2, w3, b3,
                 g1, be1, g2, be2):
    """Host-side shard + layout prep. Returns in_maps for run_bass_kernel_spmd."""
    bf = lambda a: np.ascontiguousarray(a).astype(BF16)
    f32 = lambda a: np.ascontiguousarray(a, dtype=np.float32)

    # fold LN gammas into the consuming weights (betas are zero)
    wq_f = wq * g1[None, :, None]
    wk_f = wk * g1[None, :, None]
    wv_f = wv * g1[None, :, None]
    w1_f = w1 * g2[:, None]
    w2_f = w2 * g2[:, None]

    w1img = bf(w1_f.reshape(g.EK, 128, g.NDF, 128).transpose(2, 1, 0, 3))
    w2img = bf(w2_f.reshape(g.EK, 128, g.NDF, 128).transpose(2, 1, 0, 3))
    w3img = bf(w3.reshape(g.NDF, 128, g.E))
    b1img = f32(b1.reshape(g.NDF, 128).T)
    b2img = f32(b2.reshape(g.NDF, 128).T)
    boh = bf((bo / 2.0)[None, :])
    b3img = bf(b3[None, :])
    ident = np.eye(128, dtype=BF16)

    in_maps = []
    for c in range(g.n_cores):
        b, hg = c // 2, c % 2
        hsl = slice(hg * g.HL, (hg + 1) * g.HL)

        def proj_img(w):
            # [HL, E, DH] -> SBUF image [128, EK, HL*DH]
            wl = np.transpose(w[hsl], (1, 0, 2)).reshape(g.E, g.HD)
            return bf(wl.reshape(g.EK, 128, g.HD).transpose(1, 0, 2))

        wo_l = wo[hg * g.HD:(hg + 1) * g.HD, :]
        wo_img = bf(wo_l.reshape(g.HD // 128, 128, g.E).transpose(1, 0, 2))
        in_maps.append({
            "x_full": f32(x[b]),
            "x_res": f32(x[b][hg * g.SH:(hg + 1) * g.SH]),
            "wq": proj_img(wq_f), "wk": proj_img(wk_f), "wv": proj_img(wv_f),
            "wo": wo_img,
            "w1": w1img, "w2": w2img, "w3": w3img,
            "b1": b1img, "b2": b2img, "bo_half": boh, "b3": b3img,
            "ident": ident,
        })
    return in_maps


def assemble_output(g: Geom, results):
    out = np.empty((g.B, g.S, g.E), dtype=np.float32)
    for c in range(g.n_cores):
        b, hg = c // 2, c % 2
        out[b, hg * g.SH:(hg + 1) * g.SH] = results[c]["out"]
    return out


_CACHE = {}


def kernel(**inputs) -> np.ndarray:
    g = FULL
    if "nc" not in _CACHE:
        _CACHE["nc"] = build_program(g)
    nc = _CACHE["nc"]
    in_maps = make_in_maps(g, **{k: np.asarray(v) for k, v in inputs.items()})
    res = run_bass_kernel_spmd(nc, in_maps, core_ids=list(range(g.n_cores)))
    return assemble_output(g, res.results)
